# revision 25
# baseline (speedup 1.0000x reference)
"""Gated spiking reservoir step — Trainium2 Bass kernel (8 NeuronCores).

Math (per reference):
    ic   = inputs @ input_weights                  # [B, R]
    rc   = reservoir_state @ reservoir_weights     # [B, R]
    gate = sigmoid(inputs @ gate_weights)          # [B, R]
    ns   = (0.9 * reservoir_state + 0.1 * tanh(ic + rc)) * gate
    out  = (ns > 0.5) ? 1.0 : 0.0
    returns (out, ns)

Sharding: 2D — batch split 2 ways x reservoir(column) split 4 ways.  Each of
the 8 cores owns a [1024-batch x 1024-column] block of the outputs, holding
the matching 1024-column slice of all three weight matrices and the
activations for its batch half.

All matmuls run in fp8(e4m3) DoubleRow mode: weights are prescaled by 64 on
the host (keeps the 0.02-std weights out of fp8-subnormal range) and the
1/64 is folded into the activation() scale when reading PSUM.  DoubleRow
contracts 256 rows per matmul at 0.5 PE cycles per moving element — the
per-core PE floor is ~41us and the kernel is scheduled to keep the PE
data-fed and gapless from the first real matmul to the last:

  * Host packs wg/wi column-tile-major ([128, ct, 8, 128]) so the DMA
    stream can deliver small column-pair slices in exactly the order the
    PE consumes them; the first gate matmul starts ~4us in (DMA engine
    startup + 0.375MB of operands), with warmup matmuls on a small
    scratch tile covering the p-state ramp before that.
  * Front phase: per-column gate groups in two k-half rounds (round 1
    needs only the first half of x/wg) interleaved with input-weight (ic)
    groups; sigmoid retires gates to bf16 SBUF, DVE copies retire ic into
    the bf16 accumulator.  Interleaving keeps ACT sigmoid serialization
    off the PE critical path.
  * Reservoir contraction in KS=(9,7) DoubleRow k-tile chunks.  Each
    chunk's HBM tensor is DMA'd as four need-aligned slices (s rows for
    batch half 0, w_res columns 0-3, w_res columns 4-7, s rows batch
    half 1) matching the group-block order, so each block of 4 groups
    starts as soon as ITS slice lands instead of waiting a whole-chunk
    DMA semaphore (this was the old critical path).
  * Last chunk: slots g>=SEED_G are seeded with an identity matmul over
    the accumulator so tanh reads PSUM directly and the kernel tail has
    no vector-engine retire; earlier slots retire on DVE.  The bf16 u/ns
    epilogue is software-pipelined one pair behind the tanh, each pair's
    output draining immediately as a 0.26MB DMA.  The flush queue is
    fully drained before the final pair so its half-batch closing chains
    hit an idle DVE; s9 (the 9*s bf16 term) is DMA'd in batch halves
    interleaved into the last chunk's slices so flushes never wait on the
    input stream tail.

The device computes ns' = (9*s + tanh)*gate = 10*ns — the host passes 9*s
(bf16) and divides the output by 10.  This keeps the whole epilogue in
plain tensor_tensor ops, which the DVE runs in its fast bf16 mode.
Accuracy: fp8 noise lands almost entirely inside tanh/sigmoid (the
dominant 0.9*s term uses the bf16 s9 copy), ~1.1e-2 relative error on
new_state; elements within +/-SPIKE_FIX of the 0.5 spike threshold are
recomputed exactly on the host and patched.
"""

import os
import sys

if "/opt/trn_rl_repo" not in sys.path:
    sys.path.insert(0, "/opt/trn_rl_repo")

import numpy as np

B, D_IN, R = 2048, 1024, 4096
N_CORES = 8
BP, CP = 2, 4                 # batch shards x column shards
BH = B // BP                  # 1024 batch rows per core
COLS = R // CP                # 1024 output columns per core
P = 128                       # SBUF/PSUM partitions
NB = 512                      # batch free-dim per matmul / PSUM bank
CT = COLS // P                # 8 column tiles per core
BT = BH // NB                 # 2 batch slices per core
JT = CT // 2                  # 4 fused column-tile pairs
KD = D_IN // 256              # 4 DoubleRow k-tiles over the input dim
# Reservoir-contraction chunk sizes, in 256-row DoubleRow k-tiles (sum 16).
KS = tuple(int(v) for v in os.environ.get("BASS_KS", "7,9").split(","))

SCALE = 64.0                  # host-side weight prescale (avoids fp8 subnormals)
N_WARM = int(os.environ.get("BASS_N_WARM", "30"))
SPIKE_FIX = float(os.environ.get("BASS_SPIKE_FIX", "0.03"))
SEED_G = int(os.environ.get("BASS_SEED_G", "6"))
# Last DEFER_N pairs skip the chunked contraction and run as full-K PSUM
# groups at the end (reading the already-resident chunk tiles): no DVE
# retires on the tail, tanh straight from PSUM.
DEFER_N = int(os.environ.get("BASS_DEFER_N", "2"))

_CACHE = {}


def _build():
    from contextlib import ExitStack

    from concourse import bacc, tile
    import concourse.mybir as mybir

    f32 = mybir.dt.float32
    f8 = mybir.dt.float8e4
    bf16 = mybir.dt.bfloat16
    AF = mybir.ActivationFunctionType
    ALU = mybir.AluOpType
    DR = mybir.MatmulPerfMode.DoubleRow

    nc = bacc.Bacc(
        "TRN2", target_bir_lowering=False, debug=False, enable_asserts=False
    )

    id_p = nc.dram_tensor("id_p", [P, P], bf16, kind="ExternalInput")
    x_p = nc.dram_tensor("x_p", [P, 2 * KD, BH], f8, kind="ExternalInput")
    # wg/wi column-tile-major: [p, ct, slot, cc] so column-pair DMA slices
    # are >=512B-contiguous per partition.
    wg_p = nc.dram_tensor("wg_p", [P, CT, 2 * KD, P], f8, kind="ExternalInput")
    wi_p = nc.dram_tensor("wi_p", [P, CT, 2 * KD, P], f8, kind="ExternalInput")
    # sw chunks: dim1 [0 : 2*nk] = reservoir-state rows, [2*nk :] = w_res rows
    sw_p = [
        nc.dram_tensor(f"sw_p{i}", [P, 4 * nk, COLS], f8, kind="ExternalInput")
        for i, nk in enumerate(KS)
    ]
    s9_p = nc.dram_tensor("s9_p", [P, CT, BH], bf16, kind="ExternalInput")
    ns_p = nc.dram_tensor("ns_p", [P, CT, BH], bf16, kind="ExternalOutput")

    with tile.TileContext(nc) as tc, ExitStack() as ctx:
        wpool = ctx.enter_context(tc.tile_pool(name="inputs", bufs=1))
        id_sb = wpool.tile([P, P], bf16, tag="id", name="id_sb")
        x_sb = [
            wpool.tile([P, 2 * KD, NB], f8, tag=f"x{b}", name=f"x_sb{b}")
            for b in range(BT)
        ]
        wg_sb = wpool.tile([P, CT, 2 * KD, P], f8, tag="wg", name="wg_sb")
        wi_sb = wpool.tile([P, CT, 2 * KD, P], f8, tag="wi", name="wi_sb")
        sw_sb = [
            wpool.tile([P, 4 * nk, COLS], f8, tag=f"sw{i}", name=f"sw_sb{i}")
            for i, nk in enumerate(KS)
        ]
        s9_sb = wpool.tile([P, CT, BH], bf16, tag="s9", name="s9_sb")

        warm = wpool.tile([P, 2, 256], f8, tag="warm", name="warm_sb")

        # PSUM: one pool of 8 single-bank tiles handed out in strict
        # round-robin via explicit per-slot tags — reuse distance 8 keeps
        # every group's write-after-read dependency far behind its retire.
        st_psum = ctx.enter_context(tc.tile_pool(name="st_ps", bufs=1, space="PSUM"))
        ps_n = [0]

        def ps_tile(name):
            t = st_psum.tile([P, NB], f32, tag=f"s{ps_n[0] % 8}", name=name)
            ps_n[0] += 1
            return t

        epool = ctx.enter_context(tc.tile_pool(name="epilogue", bufs=1))
        g_sb = {}
        acc_sb = {}
        ns_sb = {}
        for b in range(BT):
            for j in range(JT):
                g_sb[(b, j)] = epool.tile(
                    [P, 2, NB], bf16, tag=f"g_{b}_{j}", name=f"g_{b}_{j}"
                )
                acc_sb[(b, j)] = epool.tile(
                    [P, 2, NB], bf16, tag=f"acc_{b}_{j}", name=f"acc_{b}_{j}"
                )
                ns_sb[(b, j)] = epool.tile(
                    [P, 2, NB], bf16, tag=f"ns_{b}_{j}", name=f"ns_{b}_{j}"
                )
        tpool = ctx.enter_context(tc.tile_pool(name="tmp", bufs=3))

        # --- PE warmup: keep the tensor engine continuously busy from ~0.5us
        # so the cost model's p-state ramp (and HW HAM throttle) is spent
        # inside the initial DMA window instead of on real matmuls.  Small
        # scratch tile -> fast memset -> early first warm matmul.
        nc.gpsimd.memset(warm[:], 0)
        for j in range(N_WARM):
            # own tag cycle: keeps the real groups' PSUM round-robin phase
            # independent of N_WARM
            wps = st_psum.tile([P, NB], f32, tag=f"s{j % 8}", name=f"warm_ps{j}")
            nc.tensor.matmul(
                wps[:, 0:256],
                warm[:, :, 0:P],
                warm[:],
                start=True,
                stop=True,
                perf_mode=DR,
            )

        # --- input DMAs, in consumption order (transfers serialize on the
        # DMA engines at ~360GB/s; first-needed data goes first).
        # ~0.25MB pieces: big enough that descriptor generation (625ns per
        # DMA on the single HWDGE device) stays ahead of the 360GB/s
        # transfer stream, small enough to start matmuls early.
        nc.sync.dma_start(x_sb[0][:, 0:KD, :], x_p[:, 0:KD, 0:NB])
        nc.sync.dma_start(wg_sb[:, 0:4, 0:KD, :], wg_p[:, 0:4, 0:KD, :])
        nc.sync.dma_start(x_sb[0][:, KD : 2 * KD, :], x_p[:, KD : 2 * KD, 0:NB])
        nc.sync.dma_start(wg_sb[:, 0:4, KD : 2 * KD, :], wg_p[:, 0:4, KD : 2 * KD, :])
        nc.sync.dma_start(x_sb[1][:, 0:KD, :], x_p[:, 0:KD, NB:BH])
        nc.sync.dma_start(x_sb[1][:, KD : 2 * KD, :], x_p[:, KD : 2 * KD, NB:BH])
        nc.sync.dma_start(wi_sb[:, 0:2, :, :], wi_p[:, 0:2, :, :])
        nc.sync.dma_start(wi_sb[:, 2:4, :, :], wi_p[:, 2:4, :, :])
        nc.sync.dma_start(wg_sb[:, 4:8, 0:KD, :], wg_p[:, 4:8, 0:KD, :])
        nc.sync.dma_start(wg_sb[:, 4:8, KD : 2 * KD, :], wg_p[:, 4:8, KD : 2 * KD, :])
        nc.sync.dma_start(wi_sb[:, 4:6, :, :], wi_p[:, 4:6, :, :])
        nc.sync.dma_start(wi_sb[:, 6:8, :, :], wi_p[:, 6:8, :, :])
        nc.sync.dma_start(id_sb[:], id_p[:])
        # Reservoir chunks: four need-aligned slices each, matching the
        # (b0,j01),(b0,j23),(b1,j01),(b1,j23) block order of the group loop.
        CH = COLS // 2
        last = len(KS) - 1
        for i, nk in enumerate(KS):
            sk = slice(0, 2 * nk)
            wk = slice(2 * nk, 4 * nk)
            nc.sync.dma_start(sw_sb[i][:, sk, 0:NB], sw_p[i][:, sk, 0:NB])
            nc.sync.dma_start(sw_sb[i][:, wk, 0:CH], sw_p[i][:, wk, 0:CH])
            if i == last:
                nc.sync.dma_start(s9_sb[:, :, 0:NB], s9_p[:, :, 0:NB])
            nc.sync.dma_start(sw_sb[i][:, wk, CH:COLS], sw_p[i][:, wk, CH:COLS])
            nc.sync.dma_start(sw_sb[i][:, sk, NB:BH], sw_p[i][:, sk, NB:BH])
        nc.sync.dma_start(s9_sb[:, :, NB:BH], s9_p[:, :, NB:BH])

        # --- front phase: gates (2 k-half rounds per column tile, matching
        # the half-K DMA pieces) interleaved with ic groups.  Sigmoid retires
        # each gate PSUM bank to bf16; DVE copies retire ic into acc.
        def gate_round(b, c, half, gps):
            for k in range(half * KD // 2, (half + 1) * KD // 2):
                nc.tensor.matmul(
                    gps[:],
                    wg_sb[:, c, 2 * k : 2 * k + 2, :],
                    x_sb[b][:, 2 * k : 2 * k + 2, :],
                    start=(k == 0),
                    stop=(k == KD - 1),
                    perf_mode=DR,
                )
            if half == 1:
                nc.scalar.activation(
                    g_sb[(b, c // 2)][:, c % 2, :], gps[:], AF.Sigmoid,
                    scale=1.0 / SCALE,
                )

        def ic_group(b, c):
            ps = ps_tile(f"ic_ps_{b}_{c}")
            for k in range(KD):
                nc.tensor.matmul(
                    ps[:],
                    wi_sb[:, c, 2 * k : 2 * k + 2, :],
                    x_sb[b][:, 2 * k : 2 * k + 2, :],
                    start=(k == 0),
                    stop=(k == KD - 1),
                    perf_mode=DR,
                )
            nc.vector.tensor_copy(acc_sb[(b, c // 2)][:, c % 2, :], ps[:])

        gps = {}
        for chalf in range(2):
            crange = range(4 * chalf, 4 * chalf + 4)
            for b in range(BT):
                for c in crange:
                    gps[(b, c)] = ps_tile(f"g_ps_{b}_{c}")
                    gate_round(b, c, 0, gps[(b, c)])
                for c in crange:
                    gate_round(b, c, 1, gps[(b, c)])
            for b in range(BT):
                for c in crange:
                    ic_group(b, c)

        # --- reservoir chunks.
        NPP = BT * JT
        epi_q = []

        def flush_epi(slot):
            b, j, bs, tt = slot
            uu = tpool.tile([P, 2, NB], bf16, tag="u", bufs=4, name=f"u_{b}_{j}")
            nc.vector.tensor_tensor(
                uu[:], s9_sb[:, 2 * j : 2 * j + 2, bs], tt[:], ALU.add
            )
            nc.vector.tensor_tensor(
                ns_sb[(b, j)][:], uu[:], g_sb[(b, j)][:], ALU.mult
            )
            nc.sync.dma_start(ns_p[:, 2 * j : 2 * j + 2, bs], ns_sb[(b, j)][:])

        NCH = NPP - DEFER_N     # pairs that go through the chunked contraction
        for i, nk in enumerate(KS):
            for pp in range(NCH):
                b, j = divmod(pp, JT)
                bs = slice(b * NB, (b + 1) * NB)
                acc = acc_sb[(b, j)]
                for h in range(2):
                    c = 2 * j + h
                    cs = slice(c * P, (c + 1) * P)
                    g = 2 * pp + h          # group index within the chunk
                    seed = i == last and g >= SEED_G
                    ps = ps_tile(f"rc_ps_{i}_{b}_{c}")
                    if seed:
                        # seed PSUM with the accumulator via an identity
                        # matmul so tanh reads PSUM directly — no vector-
                        # engine retire between the group and its tanh.
                        nc.tensor.matmul(
                            ps[:],
                            id_sb[:],
                            acc[:, h, :],
                            start=True,
                            stop=False,
                        )
                    for k in range(nk):
                        nc.tensor.matmul(
                            ps[:],
                            sw_sb[i][:, 2 * nk + 2 * k : 2 * nk + 2 * k + 2, cs],
                            sw_sb[i][:, 2 * k : 2 * k + 2, bs],
                            start=(not seed and k == 0),
                            stop=(k == nk - 1),
                            perf_mode=DR,
                        )
                    if i != last:
                        # retire: acc += psum — only the DVE can both read
                        # PSUM and add tensors.
                        nc.vector.tensor_tensor(acc[:, h, :], ps[:],
                                                acc[:, h, :], ALU.add)
                    else:
                        if not seed:
                            nc.vector.tensor_tensor(acc[:, h, :], ps[:],
                                                    acc[:, h, :], ALU.add)
                        if h == 0:
                            tp_cur = tpool.tile(
                                [P, 2, NB], bf16, tag="t", bufs=4,
                                name=f"t_{b}_{j}",
                            )
                        nc.scalar.activation(
                            tp_cur[:, h, :], ps[:] if seed else acc[:, h, :],
                            AF.Tanh, scale=1.0 / SCALE,
                        )
                        if h == 1:
                            epi_q.append((b, j, bs, tp_cur))
                            if len(epi_q) > 1:
                                flush_epi(epi_q.pop(0))

        # --- deferred pairs: full-K PSUM groups (ic via identity-matmul seed
        # over acc, then all 16 reservoir k-tiles read across the resident
        # chunk tiles).  No DVE retire anywhere near the kernel tail; tanh
        # reads PSUM directly.
        for dp in range(DEFER_N):
            pp = NCH + dp
            b, j = divmod(pp, JT)
            bs = slice(b * NB, (b + 1) * NB)
            acc = acc_sb[(b, j)]
            if pp == NPP - 1:
                # drain the flush pipeline before the final pair so its
                # closing chains hit an idle DVE
                while epi_q:
                    flush_epi(epi_q.pop(0))
            for h in range(2):
                c = 2 * j + h
                cs = slice(c * P, (c + 1) * P)
                if pp == NPP - 1:
                    # final pair: h=0 stays one 512-wide group (its tanh
                    # drains early); h=1 splits into two 256-wide PSUM
                    # sub-groups (own bank each — Tile deps are per-tile, a
                    # shared bank would serialize q1's matmuls behind q0's
                    # tanh).  ACT is strictly in-order, so fewer + smaller
                    # final tanhs directly shorten the tail.
                    sub = [(slice(0, NB), NB)] if h == 0 else [
                        (slice(0, 256), 256), (slice(256, NB), 256)]
                else:
                    sub = [(slice(0, NB), NB)]
                for qn, (qs, qw) in enumerate(sub):
                    bq = slice(b * NB + qs.start, b * NB + qs.stop)
                    psq = ps_tile(f"df_ps_{pp}_{h}_{qn}")
                    nc.tensor.matmul(
                        psq[:, 0:qw], id_sb[:], acc[:, h, qs],
                        start=True, stop=False,
                    )
                    for i, nk in enumerate(KS):
                        for k in range(nk):
                            nc.tensor.matmul(
                                psq[:, 0:qw],
                                sw_sb[i][:, 2 * nk + 2 * k : 2 * nk + 2 * k + 2, cs],
                                sw_sb[i][:, 2 * k : 2 * k + 2, bq],
                                start=False,
                                stop=(i == last and k == nk - 1),
                                perf_mode=DR,
                            )
                    if pp == NPP - 1:
                        tq = tpool.tile(
                            [P, qw], bf16, tag=f"tf{h}{qn}", bufs=1,
                            name=f"tf_{h}_{qn}",
                        )
                        nc.scalar.activation(
                            tq[:], psq[:, 0:qw], AF.Tanh, scale=1.0 / SCALE,
                        )
                        uu = tpool.tile(
                            [P, qw], bf16, tag=f"uf{h}{qn}", bufs=1,
                            name=f"uf_{h}_{qn}",
                        )
                        nc.vector.tensor_tensor(
                            uu[:], s9_sb[:, c, bq], tq[:], ALU.add
                        )
                        nc.vector.tensor_tensor(
                            ns_sb[(b, j)][:, h, qs], uu[:],
                            g_sb[(b, j)][:, h, qs], ALU.mult,
                        )
                    else:
                        if h == 0 and qn == 0:
                            tp_cur = tpool.tile(
                                [P, 2, NB], bf16, tag="t", bufs=4,
                                name=f"t_{b}_{j}",
                            )
                        nc.scalar.activation(
                            tp_cur[:, h, :], psq[:, 0:qw],
                            AF.Tanh, scale=1.0 / SCALE,
                        )
                if pp == NPP - 1:
                    # one DMA per half: HWDGE serializes descriptor gen at
                    # 625ns per DMA, so fewer, not smaller
                    nc.sync.dma_start(
                        ns_p[:, c, bs], ns_sb[(b, j)][:, h, :]
                    )
                elif h == 1:
                    epi_q.append((b, j, bs, tp_cur))
                    if len(epi_q) > 1:
                        flush_epi(epi_q.pop(0))
        while epi_q:
            flush_epi(epi_q.pop(0))

    nc.compile()
    return nc


def _get_program():
    if "nc" not in _CACHE:
        _CACHE["nc"] = _build()
    return _CACHE["nc"]


def _pack_k(m):
    """[K, N] -> [128, K//128, N] DoubleRow operand layout (contraction row
    r = 256*kt + 128*i + p lives at [p, 2*kt+i, :])."""
    k, n = m.shape
    return np.ascontiguousarray(
        m.reshape(k // 256, 2, P, n).transpose(2, 0, 1, 3).reshape(P, k // P, n)
    )


def _pack_k_ct(m):
    """[K, C] -> [128, C//128, K//128, 128]: _pack_k then column-tile-major."""
    k, c = m.shape
    km = m.reshape(k // 256, 2, P, c).transpose(2, 0, 1, 3).reshape(P, k // P, c)
    return np.ascontiguousarray(
        km.reshape(P, k // P, c // P, P).transpose(0, 2, 1, 3)
    )


def _pack_ct(m):
    """[C, N] -> [128, C//128, N] plain col-tile layout (row c = 128*ct + p)."""
    c, n = m.shape
    return np.ascontiguousarray(m.reshape(c // P, P, n).transpose(1, 0, 2))


def kernel(inputs, prev_output, reservoir_state, input_weights, reservoir_weights,
           gate_weights):
    import ml_dtypes
    from concourse.bass_utils import run_bass_kernel_spmd

    F8 = ml_dtypes.float8_e4m3
    BF16 = ml_dtypes.bfloat16

    nc = _get_program()

    x = np.ascontiguousarray(np.asarray(inputs, dtype=np.float32))
    s = np.ascontiguousarray(np.asarray(reservoir_state, dtype=np.float32))
    w_in = np.asarray(input_weights, dtype=np.float32)
    w_res = np.asarray(reservoir_weights, dtype=np.float32)
    w_gate = np.asarray(gate_weights, dtype=np.float32)

    xT = x.T                     # [D_IN, B]
    sT = s.T                     # [R, B]

    in_maps = []
    for core in range(N_CORES):
        bh, cq = divmod(core, CP)
        bs = slice(bh * BH, (bh + 1) * BH)
        cs = slice(cq * COLS, (cq + 1) * COLS)
        w_res_c = w_res[:, cs] * SCALE       # [R, COLS]
        s_b = sT[:, bs]                      # [R, BH]
        m = {
            "id_p": np.eye(P).astype(BF16),
            "x_p": _pack_k(xT[:, bs]).astype(F8),
            "wg_p": _pack_k_ct(w_gate[:, cs] * SCALE).astype(F8),
            "wi_p": _pack_k_ct(w_in[:, cs] * SCALE).astype(F8),
            "s9_p": _pack_ct(sT[cs, bs] * 9.0).astype(BF16),
        }
        k0 = 0
        for i, nk in enumerate(KS):
            ks = slice(k0 * 256, (k0 + nk) * 256)
            k0 += nk
            m[f"sw_p{i}"] = np.ascontiguousarray(
                np.concatenate(
                    [_pack_k(s_b[ks]), _pack_k(w_res_c[ks])], axis=1
                )
            ).astype(F8)
        in_maps.append(m)

    res = run_bass_kernel_spmd(nc, in_maps, list(range(N_CORES)))

    new_state = np.empty((B, R), dtype=np.float32)
    for core in range(N_CORES):
        bh, cq = divmod(core, CP)
        blk = np.asarray(res.results[core]["ns_p"], dtype=np.float32)
        # device returned 10*ns; [128, CT, BH] -> [COLS, BH] -> [BH, COLS]
        blk = blk.transpose(1, 0, 2).reshape(COLS, BH) * 0.1
        new_state[bh * BH : (bh + 1) * BH, cq * COLS : (cq + 1) * COLS] = blk.T

    output = (new_state > 0.5).astype(np.float32)

    # fp8 matmuls + bf16 outputs leave ~1e-2 relative noise on new_state,
    # which only matters for the binary spike output near the 0.5 threshold.
    # Recompute those borderline elements (~1-2% of the tensor) exactly on
    # the host and patch both outputs.
    bi, rj = np.nonzero(np.abs(new_state - 0.5) < SPIKE_FIX)
    if bi.size:
        CHUNK = 16384
        for lo in range(0, bi.size, CHUNK):
            bb = bi[lo : lo + CHUNK]
            rr = rj[lo : lo + CHUNK]
            xg = x[bb]
            sg = s[bb]
            acc = np.einsum("ij,ji->i", xg, w_in[:, rr], optimize=True)
            acc += np.einsum("ij,ji->i", sg, w_res[:, rr], optimize=True)
            gacc = np.einsum("ij,ji->i", xg, w_gate[:, rr], optimize=True)
            gate = 1.0 / (1.0 + np.exp(-gacc))
            ns_fix = ((0.9 * s[bb, rr] + 0.1 * np.tanh(acc)) * gate).astype(
                np.float32
            )
            new_state[bb, rr] = ns_fix
            output[bb, rr] = (ns_fix > 0.5).astype(np.float32)
    return output, new_state


# revision 26
# speedup vs baseline: 1.0306x; 1.0306x over previous
"""Gated spiking reservoir step — Trainium2 Bass kernel (8 NeuronCores).

Math (per reference):
    ic   = inputs @ input_weights                  # [B, R]
    rc   = reservoir_state @ reservoir_weights     # [B, R]
    gate = sigmoid(inputs @ gate_weights)          # [B, R]
    ns   = (0.9 * reservoir_state + 0.1 * tanh(ic + rc)) * gate
    out  = (ns > 0.5) ? 1.0 : 0.0
    returns (out, ns)

Sharding: 2D — batch split 2 ways x reservoir(column) split 4 ways.  Each of
the 8 cores owns a [1024-batch x 1024-column] block of the outputs, holding
the matching 1024-column slice of all three weight matrices and the
activations for its batch half.

All matmuls run in fp8(e4m3) DoubleRow mode: weights are prescaled by 64 on
the host (keeps the 0.02-std weights out of fp8-subnormal range) and the
1/64 is folded into the activation() scale when reading PSUM.  DoubleRow
contracts 256 rows per matmul at 0.5 PE cycles per moving element — the
per-core PE floor is ~41us and the kernel is scheduled to keep the PE
data-fed and gapless from the first real matmul to the last:

  * Host packs wg/wi column-tile-major ([128, ct, 8, 128]) so the DMA
    stream can deliver small column-pair slices in exactly the order the
    PE consumes them; the first gate matmul starts ~4us in (DMA engine
    startup + 0.375MB of operands), with warmup matmuls on a small
    scratch tile covering the p-state ramp before that.
  * Front phase: per-column gate groups in two k-half rounds (round 1
    needs only the first half of x/wg) interleaved with input-weight (ic)
    groups; sigmoid retires gates to bf16 SBUF, DVE copies retire ic into
    the bf16 accumulator.  Interleaving keeps ACT sigmoid serialization
    off the PE critical path.
  * Reservoir contraction in KS=(9,7) DoubleRow k-tile chunks.  Each
    chunk's HBM tensor is DMA'd as four need-aligned slices (s rows for
    batch half 0, w_res columns 0-3, w_res columns 4-7, s rows batch
    half 1) matching the group-block order, so each block of 4 groups
    starts as soon as ITS slice lands instead of waiting a whole-chunk
    DMA semaphore (this was the old critical path).
  * Last chunk: slots g>=SEED_G are seeded with an identity matmul over
    the accumulator so tanh reads PSUM directly and the kernel tail has
    no vector-engine retire; earlier slots retire on DVE.  The bf16 u/ns
    epilogue is software-pipelined one pair behind the tanh, each pair's
    output draining immediately as a 0.26MB DMA.  The flush queue is
    fully drained before the final pair so its half-batch closing chains
    hit an idle DVE; s9 (the 9*s bf16 term) is DMA'd in batch halves
    interleaved into the last chunk's slices so flushes never wait on the
    input stream tail.

The device computes ns' = (9*s + tanh)*gate = 10*ns — the host passes 9*s
(bf16) and divides the output by 10.  This keeps the whole epilogue in
plain tensor_tensor ops, which the DVE runs in its fast bf16 mode.
Accuracy: fp8 noise lands almost entirely inside tanh/sigmoid (the
dominant 0.9*s term uses the bf16 s9 copy), ~1.1e-2 relative error on
new_state; elements within +/-SPIKE_FIX of the 0.5 spike threshold are
recomputed exactly on the host and patched.
"""

import os
import sys

if "/opt/trn_rl_repo" not in sys.path:
    sys.path.insert(0, "/opt/trn_rl_repo")

import numpy as np

B, D_IN, R = 2048, 1024, 4096
N_CORES = 8
BP, CP = 2, 4                 # batch shards x column shards
BH = B // BP                  # 1024 batch rows per core
COLS = R // CP                # 1024 output columns per core
P = 128                       # SBUF/PSUM partitions
NB = 512                      # batch free-dim per matmul / PSUM bank
CT = COLS // P                # 8 column tiles per core
BT = BH // NB                 # 2 batch slices per core
JT = CT // 2                  # 4 fused column-tile pairs
KD = D_IN // 256              # 4 DoubleRow k-tiles over the input dim
# Reservoir-contraction chunk sizes, in 256-row DoubleRow k-tiles (sum 16).
KS = tuple(int(v) for v in os.environ.get("BASS_KS", "10,6").split(","))

SCALE = 64.0                  # host-side weight prescale (avoids fp8 subnormals)
N_WARM = int(os.environ.get("BASS_N_WARM", "30"))
SPIKE_FIX = float(os.environ.get("BASS_SPIKE_FIX", "0.03"))
SEED_G = int(os.environ.get("BASS_SEED_G", "6"))
# Last DEFER_N pairs skip the chunked contraction and run as full-K PSUM
# groups at the end (reading the already-resident chunk tiles): no DVE
# retires on the tail, tanh straight from PSUM.
DEFER_N = int(os.environ.get("BASS_DEFER_N", "1"))

_CACHE = {}


def _build():
    from contextlib import ExitStack

    from concourse import bacc, tile
    import concourse.mybir as mybir

    f32 = mybir.dt.float32
    f8 = mybir.dt.float8e4
    bf16 = mybir.dt.bfloat16
    AF = mybir.ActivationFunctionType
    ALU = mybir.AluOpType
    DR = mybir.MatmulPerfMode.DoubleRow

    nc = bacc.Bacc(
        "TRN2", target_bir_lowering=False, debug=False, enable_asserts=False
    )

    id_p = nc.dram_tensor("id_p", [P, P], bf16, kind="ExternalInput")
    x_p = nc.dram_tensor("x_p", [P, 2 * KD, BH], f8, kind="ExternalInput")
    # wg/wi column-tile-major: [p, ct, slot, cc] so column-pair DMA slices
    # are >=512B-contiguous per partition.
    wg_p = nc.dram_tensor("wg_p", [P, CT, 2 * KD, P], f8, kind="ExternalInput")
    wi_p = nc.dram_tensor("wi_p", [P, CT, 2 * KD, P], f8, kind="ExternalInput")
    # sw chunks: dim1 [0 : 2*nk] = reservoir-state rows, [2*nk :] = w_res rows
    sw_p = [
        nc.dram_tensor(f"sw_p{i}", [P, 4 * nk, COLS], f8, kind="ExternalInput")
        for i, nk in enumerate(KS)
    ]
    s9_p = nc.dram_tensor("s9_p", [P, CT, BH], bf16, kind="ExternalInput")
    ns_p = nc.dram_tensor("ns_p", [P, CT, BH], bf16, kind="ExternalOutput")

    with tile.TileContext(nc) as tc, ExitStack() as ctx:
        wpool = ctx.enter_context(tc.tile_pool(name="inputs", bufs=1))
        id_sb = wpool.tile([P, P], bf16, tag="id", name="id_sb")
        x_sb = [
            wpool.tile([P, 2 * KD, NB], f8, tag=f"x{b}", name=f"x_sb{b}")
            for b in range(BT)
        ]
        wg_sb = wpool.tile([P, CT, 2 * KD, P], f8, tag="wg", name="wg_sb")
        wi_sb = wpool.tile([P, CT, 2 * KD, P], f8, tag="wi", name="wi_sb")
        sw_sb = [
            wpool.tile([P, 4 * nk, COLS], f8, tag=f"sw{i}", name=f"sw_sb{i}")
            for i, nk in enumerate(KS)
        ]
        s9_sb = wpool.tile([P, CT, BH], bf16, tag="s9", name="s9_sb")

        warm = wpool.tile([P, 2, 256], f8, tag="warm", name="warm_sb")

        # PSUM: one pool of 8 single-bank tiles handed out in strict
        # round-robin via explicit per-slot tags — reuse distance 8 keeps
        # every group's write-after-read dependency far behind its retire.
        st_psum = ctx.enter_context(tc.tile_pool(name="st_ps", bufs=1, space="PSUM"))
        ps_n = [0]

        def ps_tile(name):
            t = st_psum.tile([P, NB], f32, tag=f"s{ps_n[0] % 8}", name=name)
            ps_n[0] += 1
            return t

        epool = ctx.enter_context(tc.tile_pool(name="epilogue", bufs=1))
        g_sb = {}
        acc_sb = {}
        ns_sb = {}
        for b in range(BT):
            for j in range(JT):
                g_sb[(b, j)] = epool.tile(
                    [P, 2, NB], bf16, tag=f"g_{b}_{j}", name=f"g_{b}_{j}"
                )
                acc_sb[(b, j)] = epool.tile(
                    [P, 2, NB], bf16, tag=f"acc_{b}_{j}", name=f"acc_{b}_{j}"
                )
                ns_sb[(b, j)] = epool.tile(
                    [P, 2, NB], bf16, tag=f"ns_{b}_{j}", name=f"ns_{b}_{j}"
                )
        tpool = ctx.enter_context(tc.tile_pool(name="tmp", bufs=3))

        # --- PE warmup: keep the tensor engine continuously busy from ~0.5us
        # so the cost model's p-state ramp (and HW HAM throttle) is spent
        # inside the initial DMA window instead of on real matmuls.  Small
        # scratch tile -> fast memset -> early first warm matmul.
        nc.gpsimd.memset(warm[:], 0)
        for j in range(N_WARM):
            # own tag cycle: keeps the real groups' PSUM round-robin phase
            # independent of N_WARM
            wps = st_psum.tile([P, NB], f32, tag=f"s{j % 8}", name=f"warm_ps{j}")
            nc.tensor.matmul(
                wps[:, 0:256],
                warm[:, :, 0:P],
                warm[:],
                start=True,
                stop=True,
                perf_mode=DR,
            )

        # --- input DMAs, in consumption order (transfers serialize on the
        # DMA engines at ~360GB/s; first-needed data goes first).
        # ~0.25MB pieces: big enough that descriptor generation (625ns per
        # DMA on the single HWDGE device) stays ahead of the 360GB/s
        # transfer stream, small enough to start matmuls early.
        nc.sync.dma_start(x_sb[0][:, 0:KD, :], x_p[:, 0:KD, 0:NB])
        nc.sync.dma_start(wg_sb[:, 0:4, 0:KD, :], wg_p[:, 0:4, 0:KD, :])
        nc.sync.dma_start(x_sb[0][:, KD : 2 * KD, :], x_p[:, KD : 2 * KD, 0:NB])
        nc.sync.dma_start(wg_sb[:, 0:4, KD : 2 * KD, :], wg_p[:, 0:4, KD : 2 * KD, :])
        nc.sync.dma_start(x_sb[1][:, 0:KD, :], x_p[:, 0:KD, NB:BH])
        nc.sync.dma_start(x_sb[1][:, KD : 2 * KD, :], x_p[:, KD : 2 * KD, NB:BH])
        nc.sync.dma_start(wi_sb[:, 0:2, :, :], wi_p[:, 0:2, :, :])
        nc.sync.dma_start(wi_sb[:, 2:4, :, :], wi_p[:, 2:4, :, :])
        nc.sync.dma_start(wg_sb[:, 4:8, 0:KD, :], wg_p[:, 4:8, 0:KD, :])
        nc.sync.dma_start(wg_sb[:, 4:8, KD : 2 * KD, :], wg_p[:, 4:8, KD : 2 * KD, :])
        nc.sync.dma_start(wi_sb[:, 4:6, :, :], wi_p[:, 4:6, :, :])
        nc.sync.dma_start(wi_sb[:, 6:8, :, :], wi_p[:, 6:8, :, :])
        nc.sync.dma_start(id_sb[:], id_p[:])
        # Reservoir chunks: four need-aligned slices each, matching the
        # (b0,j01),(b0,j23),(b1,j01),(b1,j23) block order of the group loop.
        CH = COLS // 2
        last = len(KS) - 1
        for i, nk in enumerate(KS):
            sk = slice(0, 2 * nk)
            wk = slice(2 * nk, 4 * nk)
            nc.sync.dma_start(sw_sb[i][:, sk, 0:NB], sw_p[i][:, sk, 0:NB])
            nc.sync.dma_start(sw_sb[i][:, wk, 0:CH], sw_p[i][:, wk, 0:CH])
            if i == last:
                nc.sync.dma_start(s9_sb[:, :, 0:NB], s9_p[:, :, 0:NB])
            nc.sync.dma_start(sw_sb[i][:, wk, CH:COLS], sw_p[i][:, wk, CH:COLS])
            nc.sync.dma_start(sw_sb[i][:, sk, NB:BH], sw_p[i][:, sk, NB:BH])
        nc.sync.dma_start(s9_sb[:, :, NB:BH], s9_p[:, :, NB:BH])

        # --- front phase: gates (2 k-half rounds per column tile, matching
        # the half-K DMA pieces) interleaved with ic groups.  Sigmoid retires
        # each gate PSUM bank to bf16; DVE copies retire ic into acc.
        def gate_round(b, c, half, gps):
            for k in range(half * KD // 2, (half + 1) * KD // 2):
                nc.tensor.matmul(
                    gps[:],
                    wg_sb[:, c, 2 * k : 2 * k + 2, :],
                    x_sb[b][:, 2 * k : 2 * k + 2, :],
                    start=(k == 0),
                    stop=(k == KD - 1),
                    perf_mode=DR,
                )
            if half == 1:
                nc.scalar.activation(
                    g_sb[(b, c // 2)][:, c % 2, :], gps[:], AF.Sigmoid,
                    scale=1.0 / SCALE,
                )

        def ic_group(b, c):
            ps = ps_tile(f"ic_ps_{b}_{c}")
            for k in range(KD):
                nc.tensor.matmul(
                    ps[:],
                    wi_sb[:, c, 2 * k : 2 * k + 2, :],
                    x_sb[b][:, 2 * k : 2 * k + 2, :],
                    start=(k == 0),
                    stop=(k == KD - 1),
                    perf_mode=DR,
                )
            nc.vector.tensor_copy(acc_sb[(b, c // 2)][:, c % 2, :], ps[:])

        gps = {}
        for chalf in range(2):
            crange = range(4 * chalf, 4 * chalf + 4)
            for b in range(BT):
                for c in crange:
                    gps[(b, c)] = ps_tile(f"g_ps_{b}_{c}")
                    gate_round(b, c, 0, gps[(b, c)])
                for c in crange:
                    gate_round(b, c, 1, gps[(b, c)])
            for b in range(BT):
                for c in crange:
                    ic_group(b, c)

        # --- reservoir chunks.
        NPP = BT * JT
        epi_q = []

        def flush_epi(slot):
            b, j, bs, tt = slot
            uu = tpool.tile([P, 2, NB], bf16, tag="u", bufs=4, name=f"u_{b}_{j}")
            nc.vector.tensor_tensor(
                uu[:], s9_sb[:, 2 * j : 2 * j + 2, bs], tt[:], ALU.add
            )
            nc.vector.tensor_tensor(
                ns_sb[(b, j)][:], uu[:], g_sb[(b, j)][:], ALU.mult
            )
            nc.sync.dma_start(ns_p[:, 2 * j : 2 * j + 2, bs], ns_sb[(b, j)][:])

        NCH = NPP - DEFER_N     # pairs that go through the chunked contraction
        for i, nk in enumerate(KS):
            for pp in range(NCH):
                b, j = divmod(pp, JT)
                bs = slice(b * NB, (b + 1) * NB)
                acc = acc_sb[(b, j)]
                for h in range(2):
                    c = 2 * j + h
                    cs = slice(c * P, (c + 1) * P)
                    g = 2 * pp + h          # group index within the chunk
                    seed = i == last and g >= SEED_G
                    ps = ps_tile(f"rc_ps_{i}_{b}_{c}")
                    if seed:
                        # seed PSUM with the accumulator via an identity
                        # matmul so tanh reads PSUM directly — no vector-
                        # engine retire between the group and its tanh.
                        nc.tensor.matmul(
                            ps[:],
                            id_sb[:],
                            acc[:, h, :],
                            start=True,
                            stop=False,
                        )
                    for k in range(nk):
                        nc.tensor.matmul(
                            ps[:],
                            sw_sb[i][:, 2 * nk + 2 * k : 2 * nk + 2 * k + 2, cs],
                            sw_sb[i][:, 2 * k : 2 * k + 2, bs],
                            start=(not seed and k == 0),
                            stop=(k == nk - 1),
                            perf_mode=DR,
                        )
                    if i != last:
                        # retire: acc += psum — only the DVE can both read
                        # PSUM and add tensors.
                        nc.vector.tensor_tensor(acc[:, h, :], ps[:],
                                                acc[:, h, :], ALU.add)
                    else:
                        if not seed:
                            nc.vector.tensor_tensor(acc[:, h, :], ps[:],
                                                    acc[:, h, :], ALU.add)
                        if h == 0:
                            tp_cur = tpool.tile(
                                [P, 2, NB], bf16, tag="t", bufs=4,
                                name=f"t_{b}_{j}",
                            )
                        nc.scalar.activation(
                            tp_cur[:, h, :], ps[:] if seed else acc[:, h, :],
                            AF.Tanh, scale=1.0 / SCALE,
                        )
                        if h == 1:
                            epi_q.append((b, j, bs, tp_cur))
                            if len(epi_q) > 1:
                                flush_epi(epi_q.pop(0))

        # --- deferred pairs: full-K PSUM groups (ic via identity-matmul seed
        # over acc, then all 16 reservoir k-tiles read across the resident
        # chunk tiles).  No DVE retire anywhere near the kernel tail; tanh
        # reads PSUM directly.
        for dp in range(DEFER_N):
            pp = NCH + dp
            b, j = divmod(pp, JT)
            bs = slice(b * NB, (b + 1) * NB)
            acc = acc_sb[(b, j)]
            if pp == NPP - 1:
                # drain the flush pipeline before the final pair so its
                # closing chains hit an idle DVE
                while epi_q:
                    flush_epi(epi_q.pop(0))
            for h in range(2):
                c = 2 * j + h
                cs = slice(c * P, (c + 1) * P)
                if pp == NPP - 1:
                    # final pair: h=0 stays one 512-wide group (its tanh
                    # drains early); h=1 splits into two 256-wide PSUM
                    # sub-groups (own bank each — Tile deps are per-tile, a
                    # shared bank would serialize q1's matmuls behind q0's
                    # tanh).  ACT is strictly in-order, so fewer + smaller
                    # final tanhs directly shorten the tail.
                    sub = [(slice(0, NB), NB)] if h == 0 else [
                        (slice(0, 256), 256), (slice(256, NB), 256)]
                else:
                    sub = [(slice(0, NB), NB)]
                for qn, (qs, qw) in enumerate(sub):
                    bq = slice(b * NB + qs.start, b * NB + qs.stop)
                    psq = ps_tile(f"df_ps_{pp}_{h}_{qn}")
                    nc.tensor.matmul(
                        psq[:, 0:qw], id_sb[:], acc[:, h, qs],
                        start=True, stop=False,
                    )
                    for i, nk in enumerate(KS):
                        for k in range(nk):
                            nc.tensor.matmul(
                                psq[:, 0:qw],
                                sw_sb[i][:, 2 * nk + 2 * k : 2 * nk + 2 * k + 2, cs],
                                sw_sb[i][:, 2 * k : 2 * k + 2, bq],
                                start=False,
                                stop=(i == last and k == nk - 1),
                                perf_mode=DR,
                            )
                    if pp == NPP - 1:
                        tq = tpool.tile(
                            [P, qw], bf16, tag=f"tf{h}{qn}", bufs=1,
                            name=f"tf_{h}_{qn}",
                        )
                        nc.scalar.activation(
                            tq[:], psq[:, 0:qw], AF.Tanh, scale=1.0 / SCALE,
                        )
                        uu = tpool.tile(
                            [P, qw], bf16, tag=f"uf{h}{qn}", bufs=1,
                            name=f"uf_{h}_{qn}",
                        )
                        nc.vector.tensor_tensor(
                            uu[:], s9_sb[:, c, bq], tq[:], ALU.add
                        )
                        nc.vector.tensor_tensor(
                            ns_sb[(b, j)][:, h, qs], uu[:],
                            g_sb[(b, j)][:, h, qs], ALU.mult,
                        )
                    else:
                        if h == 0 and qn == 0:
                            tp_cur = tpool.tile(
                                [P, 2, NB], bf16, tag="t", bufs=4,
                                name=f"t_{b}_{j}",
                            )
                        nc.scalar.activation(
                            tp_cur[:, h, :], psq[:, 0:qw],
                            AF.Tanh, scale=1.0 / SCALE,
                        )
                if pp == NPP - 1:
                    # one DMA per half: HWDGE serializes descriptor gen at
                    # 625ns per DMA, so fewer, not smaller
                    nc.sync.dma_start(
                        ns_p[:, c, bs], ns_sb[(b, j)][:, h, :]
                    )
                elif h == 1:
                    epi_q.append((b, j, bs, tp_cur))
                    if len(epi_q) > 1:
                        flush_epi(epi_q.pop(0))
        while epi_q:
            flush_epi(epi_q.pop(0))

    nc.compile()
    return nc


def _get_program():
    if "nc" not in _CACHE:
        _CACHE["nc"] = _build()
    return _CACHE["nc"]


def _pack_k(m):
    """[K, N] -> [128, K//128, N] DoubleRow operand layout (contraction row
    r = 256*kt + 128*i + p lives at [p, 2*kt+i, :])."""
    k, n = m.shape
    return np.ascontiguousarray(
        m.reshape(k // 256, 2, P, n).transpose(2, 0, 1, 3).reshape(P, k // P, n)
    )


def _pack_k_ct(m):
    """[K, C] -> [128, C//128, K//128, 128]: _pack_k then column-tile-major."""
    k, c = m.shape
    km = m.reshape(k // 256, 2, P, c).transpose(2, 0, 1, 3).reshape(P, k // P, c)
    return np.ascontiguousarray(
        km.reshape(P, k // P, c // P, P).transpose(0, 2, 1, 3)
    )


def _pack_ct(m):
    """[C, N] -> [128, C//128, N] plain col-tile layout (row c = 128*ct + p)."""
    c, n = m.shape
    return np.ascontiguousarray(m.reshape(c // P, P, n).transpose(1, 0, 2))


def kernel(inputs, prev_output, reservoir_state, input_weights, reservoir_weights,
           gate_weights):
    import ml_dtypes
    from concourse.bass_utils import run_bass_kernel_spmd

    F8 = ml_dtypes.float8_e4m3
    BF16 = ml_dtypes.bfloat16

    nc = _get_program()

    x = np.ascontiguousarray(np.asarray(inputs, dtype=np.float32))
    s = np.ascontiguousarray(np.asarray(reservoir_state, dtype=np.float32))
    w_in = np.asarray(input_weights, dtype=np.float32)
    w_res = np.asarray(reservoir_weights, dtype=np.float32)
    w_gate = np.asarray(gate_weights, dtype=np.float32)

    xT = x.T                     # [D_IN, B]
    sT = s.T                     # [R, B]

    in_maps = []
    for core in range(N_CORES):
        bh, cq = divmod(core, CP)
        bs = slice(bh * BH, (bh + 1) * BH)
        cs = slice(cq * COLS, (cq + 1) * COLS)
        w_res_c = w_res[:, cs] * SCALE       # [R, COLS]
        s_b = sT[:, bs]                      # [R, BH]
        m = {
            "id_p": np.eye(P).astype(BF16),
            "x_p": _pack_k(xT[:, bs]).astype(F8),
            "wg_p": _pack_k_ct(w_gate[:, cs] * SCALE).astype(F8),
            "wi_p": _pack_k_ct(w_in[:, cs] * SCALE).astype(F8),
            "s9_p": _pack_ct(sT[cs, bs] * 9.0).astype(BF16),
        }
        k0 = 0
        for i, nk in enumerate(KS):
            ks = slice(k0 * 256, (k0 + nk) * 256)
            k0 += nk
            m[f"sw_p{i}"] = np.ascontiguousarray(
                np.concatenate(
                    [_pack_k(s_b[ks]), _pack_k(w_res_c[ks])], axis=1
                )
            ).astype(F8)
        in_maps.append(m)

    res = run_bass_kernel_spmd(nc, in_maps, list(range(N_CORES)))

    new_state = np.empty((B, R), dtype=np.float32)
    for core in range(N_CORES):
        bh, cq = divmod(core, CP)
        blk = np.asarray(res.results[core]["ns_p"], dtype=np.float32)
        # device returned 10*ns; [128, CT, BH] -> [COLS, BH] -> [BH, COLS]
        blk = blk.transpose(1, 0, 2).reshape(COLS, BH) * 0.1
        new_state[bh * BH : (bh + 1) * BH, cq * COLS : (cq + 1) * COLS] = blk.T

    output = (new_state > 0.5).astype(np.float32)

    # fp8 matmuls + bf16 outputs leave ~1e-2 relative noise on new_state,
    # which only matters for the binary spike output near the 0.5 threshold.
    # Recompute those borderline elements (~1-2% of the tensor) exactly on
    # the host and patch both outputs.
    bi, rj = np.nonzero(np.abs(new_state - 0.5) < SPIKE_FIX)
    if bi.size:
        CHUNK = 16384
        for lo in range(0, bi.size, CHUNK):
            bb = bi[lo : lo + CHUNK]
            rr = rj[lo : lo + CHUNK]
            xg = x[bb]
            sg = s[bb]
            acc = np.einsum("ij,ji->i", xg, w_in[:, rr], optimize=True)
            acc += np.einsum("ij,ji->i", sg, w_res[:, rr], optimize=True)
            gacc = np.einsum("ij,ji->i", xg, w_gate[:, rr], optimize=True)
            gate = 1.0 / (1.0 + np.exp(-gacc))
            ns_fix = ((0.9 * s[bb, rr] + 0.1 * np.tanh(acc)) * gate).astype(
                np.float32
            )
            new_state[bb, rr] = ns_fix
            output[bb, rr] = (ns_fix > 0.5).astype(np.float32)
    return output, new_state
